# revision 1
# baseline (speedup 1.0000x reference)
"""Distributed Trainium2 kernel for ArceeAttention (GQA + RoPE + causal attention).

Sharding: DP over batch (2 groups of 4 cores) x TP-4 over heads within each group.
Each core: 8 q heads + 2 kv heads, full sequence of its batch.
ReduceScatter(add) over each 4-core group combines o-proj partials, pipelined
per 512-token slab so communication overlaps attention of the next slab.

Structure:
  A0: hidden -> bf16 -> xbar-transposed hidT; QKV^T matmuls (w_qkv stationary)
      with RoPE fused on the way out; V in natural [tok, d] layout
      (hidT stationary). w_qkv SBUF freed afterwards.
  A1: per 512-token slab: causal attention with 512-wide query blocks
      (scores kept transposed so PV needs no transposes), softmax without
      max-subtraction (scores ~N(0,1)), denominator via ones-matmul;
      then o-proj (attn^T stationary) and ReduceScatter of bf16 partials.
"""
import sys
import numpy as np

for _p in ("/opt/trn_rl_repo",):
    if _p not in sys.path:
        sys.path.append(_p)

import ml_dtypes  # noqa: E402
from concourse import bass, bacc, tile, mybir  # noqa: E402
from concourse.bass_utils import run_bass_kernel_spmd  # noqa: E402

F32 = mybir.dt.float32
BF16 = mybir.dt.bfloat16
I32 = mybir.dt.int32

ROPE_THETA = 10000.0
D = 128  # head dim

_NC_CACHE = {}


def build_nc(S=2048, HID=4096, NQ=8, NKV=2, CHUNK=256):
    REP = NQ // NKV
    QC = NQ * D               # q cols per core
    KC = NKV * D              # k (or v) cols per core
    NCH = S // CHUNK          # token chunks in A0
    KTC = CHUNK // 128
    NHT = HID // 128          # hidden-dim tiles
    SLAB = min(512, S)        # tokens per ReduceScatter slab / query block
    NSLAB = S // SLAB
    SKT = SLAB // 128         # k-tiles per slab
    NHC = max(1, HID // 512)  # hid chunks for o-proj
    W = HID // NHC
    HG = min(2, NHC)          # o-proj psum group size
    SCALE = float(D) ** -0.5
    RG = [[0, 1, 2, 3], [4, 5, 6, 7]]

    MAGIC = 12582912.0        # 1.5 * 2**23: float32 round-to-nearest-int trick
    TWOPI = float(2.0 * np.pi)
    INV2PI = float(1.0 / TWOPI)
    HALFPI = float(np.pi / 2.0)

    nc = bacc.Bacc(None, target_bir_lowering=False)
    hidden = nc.declare_dram_parameter("hidden_states", [S, HID], F32, isOutput=False)
    positions = nc.declare_dram_parameter("positions", [1, S], I32, isOutput=False)
    w_qkv = nc.declare_dram_parameter("w_qkv", [HID, QC + 2 * KC], F32, isOutput=False)
    w_o = nc.declare_dram_parameter("w_o", [QC, HID], F32, isOutput=False)
    invf = nc.declare_dram_parameter("invfreq128", [128, 1], F32, isOutput=False)
    sign = nc.declare_dram_parameter("sign128", [128, 1], F32, isOutput=False)
    masks = nc.declare_dram_parameter("masks", [128, SKT, SLAB], BF16, isOutput=False)
    swapm = nc.declare_dram_parameter("swapmat", [128, 128], F32, isOutput=False)
    out = nc.declare_dram_parameter("out", [NSLAB, SLAB // 4, HID], F32, isOutput=True)

    Exp = mybir.ActivationFunctionType.Exp
    Sin = mybir.ActivationFunctionType.Sin
    mul_op = mybir.AluOpType.mult
    add_op = mybir.AluOpType.add

    with tile.TileContext(nc) as tc:
      with tc.tile_pool(name="dram", bufs=1, space="DRAM") as dram:
        hid_bf = [dram.tile([CHUNK, HID], BF16, name=f"hid_bf{c}", tag=f"hid_bf{c}")
                  for c in range(NCH)]
        wo_bf = dram.tile([QC, HID], BF16, name="wo_bf", tag="wo_bf")
        parts = [dram.tile([SLAB, HID], BF16, name=f"part{s}", tag=f"part{s}")
                 for s in range(NSLAB)]
        rsouts = [dram.tile([SLAB // 4, HID], BF16, name=f"rsout{s}", tag=f"rsout{s}")
                  for s in range(NSLAB)]

        with tc.tile_pool(name="const", bufs=1) as cpool:
            mask_sb = cpool.tile([128, SKT, SLAB], BF16, name="masks", tag="masks")
            nc.sync.dma_start(mask_sb[:], masks[:])
            invf_sb = cpool.tile([128, 1], F32, name="invf", tag="invf")
            nc.sync.dma_start(invf_sb[:], invf[:])
            sign_sb = cpool.tile([128, 1], F32, name="sign", tag="sign")
            nc.sync.dma_start(sign_sb[:], sign[:])
            ones_col = cpool.tile([128, 1], BF16, name="ones_col", tag="ones_col")
            nc.vector.memset(ones_col[:], 1.0)
            ones_row = cpool.tile([1, 128], F32, name="ones_row", tag="ones_row")
            nc.vector.memset(ones_row[:], 1.0)
            swap_sb = cpool.tile([128, 128], F32, name="swapm", tag="swapm")
            nc.sync.dma_start(swap_sb[:], swapm[:])

            # persistent across A0 -> A1
            with tc.tile_pool(name="qkv_keep", bufs=1) as kvp:
                qT_all = [kvp.tile([128, S], BF16, name=f"qT{i}", tag=f"qT{i}")
                          for i in range(NQ)]
                kT_sb = [kvp.tile([128, S], BF16, name=f"kT{i}", tag=f"kT{i}")
                         for i in range(NKV)]
                v_sb = [kvp.tile([128, KC], BF16, name=f"v{t}", tag=f"v{t}")
                        for t in range(S // 128)]

                # ================= A0: QKV + RoPE =================
                with (
                    tc.tile_pool(name="wq", bufs=1) as wqp,
                    tc.tile_pool(name="hidT", bufs=2) as hTp,
                    tc.tile_pool(name="rope", bufs=2) as rp,
                    tc.tile_pool(name="trigc", bufs=2) as tgp,
                    tc.tile_pool(name="psA", bufs=3, space="PSUM") as psA,
                    tc.tile_pool(name="psT", bufs=2, space="PSUM") as psT,
                ):
                    wq_sb = [wqp.tile([128, QC + 2 * KC], BF16,
                                      name=f"wq{h}", tag=f"wq{h}")
                             for h in range(NHT)]
                    def emit_a1dma(c):
                        for j in range(4):
                            r0 = (CHUNK // 4) * j
                            nc.gpsimd.dma_start(
                                hid_bf[c][r0:r0 + CHUNK // 4, :],
                                hidden[CHUNK * c + r0:CHUNK * c + r0 + CHUNK // 4, :],
                            )

                    for c in range(min(3, NCH)):
                        emit_a1dma(c)
                    for h in range(NHT):
                        nc.gpsimd.dma_start(
                            wq_sb[h][:], w_qkv[128 * h:128 * (h + 1), :]
                        )  # f32 -> bf16 cast in DMA (SWDGE)

                    def emit_transposes(c):
                        tiles = [hTp.tile([128, CHUNK], BF16,
                                          name=f"hidT{h}", tag=f"hidT{h}")
                                 for h in range(NHT)]
                        for h in range(NHT):
                            nc.sync.dma_start_transpose(
                                tiles[h][:], hid_bf[c][:, 128 * h:128 * (h + 1)]
                            )
                        return tiles

                    hidT = emit_transposes(0)
                    for c in range(NCH):
                        c0 = CHUNK * c
                        for t in range(3 + 2 * c, min(5 + 2 * c, NCH)):
                            emit_a1dma(t)
                        hidT_next = emit_transposes(c + 1) if c + 1 < NCH else None
                        # smear the w_o pre-cast across late chunks (casts done)
                        den_ch = max(1, NCH - 3)
                        wo_lo = (c - 3) * NQ // den_ch if c >= 3 else 0
                        wo_hi = (c - 2) * NQ // den_ch if c >= 3 else 0
                        if c == NCH - 1:
                            wo_hi = NQ
                        for q in range(wo_lo, wo_hi):
                            nc.gpsimd.dma_start(
                                wo_bf[128 * q:128 * (q + 1), :],
                                w_o[128 * q:128 * (q + 1), :],
                            )
                        # per-chunk cos/sin [128, CHUNK]
                        pos_i = tgp.tile([1, CHUNK], I32, name="posi", tag="posi")
                        nc.gpsimd.dma_start(pos_i[:], positions[0:1, c0:c0 + CHUNK])
                        pos_c = tgp.tile([1, CHUNK], F32, name="posc", tag="posc")
                        nc.vector.tensor_copy(pos_c[:], pos_i[:])
                        ppos = psT.tile([128, CHUNK], F32, name="ppos", tag="ppos")
                        nc.tensor.matmul(ppos[:], ones_row[:], pos_c[:],
                                         start=True, stop=True)
                        ang = tgp.tile([128, CHUNK], F32, name="ang", tag="ang")
                        nc.vector.tensor_scalar_mul(ang[:], ppos[:], invf_sb[:])
                        cosc = tgp.tile([128, CHUNK], F32, name="cosc", tag="cosc")
                        sinc = tgp.tile([128, CHUNK], F32, name="sinc", tag="sinc")
                        tmp = tgp.tile([128, CHUNK], F32, name="ttmp", tag="ttmp")
                        red = tgp.tile([128, CHUNK], F32, name="tred", tag="tred")
                        for dst, phase in ((cosc, HALFPI), (sinc, 0.0)):
                            nc.vector.tensor_scalar(
                                tmp[:], ang[:], INV2PI, phase * INV2PI,
                                op0=mul_op, op1=add_op)
                            nc.vector.tensor_scalar_add(tmp[:], tmp[:], MAGIC)
                            nc.vector.tensor_scalar_sub(tmp[:], tmp[:], MAGIC)
                            nc.vector.scalar_tensor_tensor(
                                red[:], tmp[:], -TWOPI, ang[:],
                                op0=mul_op, op1=add_op)
                            if phase != 0.0:
                                nc.vector.tensor_scalar_add(red[:], red[:], phase)
                            nc.vector.tensor_scalar_min(red[:], red[:], 3.141592)
                            nc.vector.tensor_scalar_max(red[:], red[:], -3.141592)
                            nc.scalar.activation(dst[:], red[:], Sin)
                        nc.vector.tensor_scalar_mul(sinc[:], sinc[:], sign_sb[:])

                        # Q^T / K^T col-tiles + RoPE
                        for ct in range(NQ + NKV):
                            pq = psA.tile([128, CHUNK], F32, name="pq", tag="pq")
                            for h in range(NHT):
                                nc.tensor.matmul(
                                    pq[:],
                                    wq_sb[h][:, 128 * ct:128 * (ct + 1)],
                                    hidT[h][:],
                                    start=(h == 0), stop=(h == NHT - 1),
                                )
                            qf = rp.tile([128, CHUNK], F32, name="qf", tag="qf")
                            nc.scalar.copy(qf[:], pq[:])
                            pswap = psT.tile([128, CHUNK], F32, name="pswap", tag="ppos")
                            nc.tensor.matmul(pswap[:], swap_sb[:], qf[:],
                                             start=True, stop=True)
                            rot = rp.tile([128, CHUNK], F32, name="rot", tag="rot")
                            nc.vector.tensor_mul(rot[:], pswap[:], sinc[:])
                            tc2 = rp.tile([128, CHUNK], F32, name="tc2", tag="tc2")
                            nc.vector.tensor_mul(tc2[:], qf[:], cosc[:])
                            if ct < NQ:
                                dst = qT_all[ct][:, c0:c0 + CHUNK]
                            else:
                                dst = kT_sb[ct - NQ][:, c0:c0 + CHUNK]
                            nc.vector.tensor_add(dst, tc2[:], rot[:])
                        # V natural [tok, d]
                        for tt in range(KTC):
                            pv = psA.tile([128, KC], F32, name="pv", tag="pq")
                            for h in range(NHT):
                                nc.tensor.matmul(
                                    pv[:],
                                    hidT[h][:, 128 * tt:128 * (tt + 1)],
                                    wq_sb[h][:, QC + KC:QC + 2 * KC],
                                    start=(h == 0), stop=(h == NHT - 1),
                                )
                            nc.scalar.copy(v_sb[c * KTC + tt][:], pv[:])
                        hidT = hidT_next

                # ============ A1: attention + o-proj + RS ============
                with (
                    tc.tile_pool(name="wo", bufs=1) as wop,
                    tc.tile_pool(name="at", bufs=2) as atp,
                    tc.tile_pool(name="pt", bufs=4) as ptp,
                    tc.tile_pool(name="den", bufs=2) as dnp,
                    tc.tile_pool(name="bcp", bufs=2) as bcp,
                    tc.tile_pool(name="ost", bufs=4) as ostp,
                    tc.tile_pool(name="psS", bufs=2, space="PSUM") as psS,
                    tc.tile_pool(name="psPV", bufs=3, space="PSUM") as psPV,
                    tc.tile_pool(name="psX", bufs=1, space="PSUM") as psX,
                    tc.tile_pool(name="psO", bufs=1, space="PSUM") as psO,
                ):
                    wo_sb = [wop.tile([128, HID], BF16, name=f"wo{q}", tag=f"wo{q}")
                             for q in range(NQ)]
                    for q in range(NQ):
                        nc.sync.dma_start(wo_sb[q][:], wo_bf[128 * q:128 * (q + 1), :])

                    for s in range(NSLAB):
                        s0 = SLAB * s
                        NKT = (s + 1) * SKT
                        at_tiles = []
                        for hq in range(NQ):
                            kvh = hq // REP
                            ppv = psPV.tile([128, SLAB], F32, name="ppv", tag="ppv")
                            den = dnp.tile([128, SLAB], BF16, name="den", tag="den")
                            for kt in range(NKT):
                                ps = psS.tile([128, SLAB], F32, name="ps", tag="ps")
                                nc.tensor.matmul(
                                    ps[:],
                                    kT_sb[kvh][:, 128 * kt:128 * (kt + 1)],
                                    qT_all[hq][:, s0:s0 + SLAB],
                                    start=True, stop=True,
                                )
                                pt = ptp.tile([128, SLAB], BF16, name="pt", tag="pt")
                                nc.scalar.activation(pt[:], ps[:], Exp, scale=SCALE)
                                diag = kt - s * SKT
                                if diag >= 0:
                                    nc.vector.tensor_mul(
                                        pt[:], pt[:], mask_sb[:, diag, :])
                                if kt == 0:
                                    nc.vector.tensor_copy(den[:], pt[:])
                                else:
                                    nc.vector.tensor_add(den[:], den[:], pt[:])
                                nc.tensor.matmul(
                                    ppv[:],
                                    v_sb[kt][:, D * kvh:D * (kvh + 1)],
                                    pt[:],
                                    start=(kt == 0), stop=(kt == NKT - 1),
                                )
                            pden = psX.tile([128, SLAB], F32, name="pden", tag="pden")
                            nc.tensor.matmul(pden[0:1, :], ones_col[:], den[:],
                                             start=True, stop=True)
                            rec = dnp.tile([1, SLAB], F32, name="rec", tag="rec")
                            nc.vector.reciprocal_approx_fast(rec[:], pden[0:1, :])
                            pbc = psX.tile([128, SLAB], F32, name="pbc", tag="pden")
                            nc.tensor.matmul(pbc[:], ones_row[:], rec[:],
                                             start=True, stop=True)
                            bc = bcp.tile([128, SLAB], F32, name="bc", tag="bc")
                            nc.vector.tensor_copy(bc[:], pbc[:])
                            at = atp.tile([128, SLAB], BF16,
                                          name=f"at{hq}", tag=f"at{hq}")
                            nc.vector.tensor_mul(at[:], ppv[:], bc[:])
                            at_tiles.append(at)
                        # o-proj for this slab
                        for tt in range(SLAB // 128):
                            for hg in range(NHC // HG):
                                pos = [psO.tile([128, W], F32,
                                                name=f"po{j}", tag=f"po{j}")
                                       for j in range(HG)]
                                for q in range(NQ):
                                    for j in range(HG):
                                        hc = hg * HG + j
                                        nc.tensor.matmul(
                                            pos[j][:],
                                            at_tiles[q][:, 128 * tt:128 * (tt + 1)],
                                            wo_sb[q][:, W * hc:W * (hc + 1)],
                                            start=(q == 0), stop=(q == NQ - 1),
                                        )
                                for j in range(HG):
                                    hc = hg * HG + j
                                    ot = ostp.tile([128, W], BF16,
                                                   name="ot", tag="ot")
                                    nc.vector.tensor_copy(ot[:], pos[j][:])
                                    nc.sync.dma_start(
                                        parts[s][128 * tt:128 * (tt + 1),
                                                 W * hc:W * (hc + 1)],
                                        ot[:],
                                    )
                        nc.gpsimd.collective_compute(
                            "ReduceScatter",
                            mybir.AluOpType.add,
                            replica_groups=RG,
                            ins=[parts[s].opt()],
                            outs=[rsouts[s].opt()],
                        )
                        nc.gpsimd.dma_start(out[s], rsouts[s][:])  # bf16 -> f32

    nc.compile()
    return nc


def make_consts(S=2048):
    SLAB = min(512, S)
    SKT = SLAB // 128
    d_half = np.arange(0, D, 2, dtype=np.float32) / D
    invfreq = (1.0 / (ROPE_THETA ** d_half)).astype(np.float32)  # [64]
    invf128 = np.concatenate([invfreq, invfreq]).reshape(128, 1).astype(np.float32)
    sign128 = np.concatenate([
        -np.ones(64, np.float32), np.ones(64, np.float32)
    ]).reshape(128, 1)
    p = np.arange(128).reshape(128, 1, 1)
    j = np.arange(SKT).reshape(1, SKT, 1)
    q = np.arange(SLAB).reshape(1, 1, SLAB)
    masks = ((j * 128 + p) <= q).astype(ml_dtypes.bfloat16)  # [128, SKT, SLAB]
    swapmat = np.zeros((128, 128), np.float32)
    for pp in range(128):
        swapmat[pp, (pp + 64) % 128] = 1.0
    return invf128, sign128, masks, swapmat


def shard_inputs(hidden_states, positions, w_qkv, w_o, n_q_total=32, n_kv_total=8,
                 tp=4):
    """Returns in_maps for 8 cores: DP over batch x TP over heads."""
    B, S, HIDDEN = hidden_states.shape
    q_size = n_q_total * D
    kv_size = n_kv_total * D
    nq = n_q_total // tp           # q heads per core
    nkv = n_kv_total // tp         # kv heads per core
    invf128, sign128, masks, swapmat = make_consts(S=S)
    in_maps = []
    for c in range(8):
        g, r = divmod(c, tp)
        wq = w_qkv[:, nq * D * r: nq * D * (r + 1)]
        wk = w_qkv[:, q_size + nkv * D * r: q_size + nkv * D * (r + 1)]
        wv = w_qkv[:, q_size + kv_size + nkv * D * r: q_size + kv_size + nkv * D * (r + 1)]
        in_maps.append({
            "hidden_states": np.ascontiguousarray(hidden_states[g]),
            "positions": np.ascontiguousarray(positions[g:g + 1]).astype(np.int32),
            "w_qkv": np.ascontiguousarray(np.concatenate([wq, wk, wv], axis=1)),
            "w_o": np.ascontiguousarray(w_o[nq * D * r: nq * D * (r + 1), :]),
            "invfreq128": invf128,
            "sign128": sign128,
            "masks": masks,
            "swapmat": swapmat,
        })
    return in_maps


def assemble_output(results, B=2, S=2048, HIDDEN=4096, tp=4):
    SLAB = min(512, S)
    NSLAB = S // SLAB
    RSROWS = SLAB // 4
    out = np.empty((B, S, HIDDEN), dtype=np.float32)
    for c in range(8):
        g, r = divmod(c, tp)
        o = np.asarray(results[c]["out"]).reshape(NSLAB, RSROWS, HIDDEN)
        for s in range(NSLAB):
            t0 = SLAB * s + RSROWS * r
            out[g, t0:t0 + RSROWS, :] = o[s]
    return out


def kernel(hidden_states, positions, w_qkv, w_o):
    hidden_states = np.asarray(hidden_states, dtype=np.float32)
    positions = np.asarray(positions, dtype=np.int32)
    w_qkv = np.asarray(w_qkv, dtype=np.float32)
    w_o = np.asarray(w_o, dtype=np.float32)
    B, S, HIDDEN = hidden_states.shape

    key = (S, HIDDEN)
    if key not in _NC_CACHE:
        _NC_CACHE[key] = build_nc(S=S, HID=HIDDEN)
    nc = _NC_CACHE[key]

    in_maps = shard_inputs(hidden_states, positions, w_qkv, w_o)
    res = run_bass_kernel_spmd(nc, in_maps, core_ids=list(range(8)))
    return assemble_output(res.results, B=B, S=S, HIDDEN=HIDDEN)


if __name__ == "__main__":
    rng = np.random.default_rng(0)
    B, S, HIDDEN = 2, 2048, 4096
    hs = rng.standard_normal((B, S, HIDDEN), dtype=np.float32)
    pos = np.arange(B * S, dtype=np.int32).reshape(B, S)
    wq = rng.standard_normal((HIDDEN, 6144), dtype=np.float32) * HIDDEN ** -0.5
    wo = rng.standard_normal((4096, HIDDEN), dtype=np.float32) * 4096 ** -0.5
    o = kernel(hs, pos, wq, wo)
    print(o.shape, o.dtype)



# revision 3
# speedup vs baseline: 1.0177x; 1.0177x over previous
"""Distributed Trainium2 kernel for ArceeAttention (GQA + RoPE + causal attention).

Sharding: DP over batch (2 groups of 4 cores) x TP-4 over heads within each group.
Each core: 8 q heads + 2 kv heads, full sequence of its batch.

Structure (v2 — engine-dense rework of the baseline):
  A0: hidden -> bf16 DRAM staging -> xbar transposes [512,128] on BOTH HWDGE
      queues (sync+scalar); chunk-major QKV with N=512 moving operand and the
      full w_qkv resident bf16; RoPE trig precomputed once for the whole
      sequence (sign folded into invfreq); RoPE swap matmul in float32r
      (1 cyc/row); qT/kT/v staged to DRAM (keeps SBUF under budget).
  A1: per 512-token slab: scores in [128, 2x512] PSUM supertiles, ONE exp per
      supertile, denominator accumulated on DVE as bf16 supertiles and folded
      by ones-matmuls; o-proj of slab s-1 interleaved with attention of slab s
      so the PE never drains (HAM stays warm) while ACT chews exps;
      ReduceScatter chunked per 128 tokens (16 ops) so the tail is short.
"""
import sys
import numpy as np

for _p in ("/opt/trn_rl_repo",):
    if _p not in sys.path:
        sys.path.append(_p)

import ml_dtypes  # noqa: E402
from concourse import bass, bacc, tile, mybir  # noqa: E402
from concourse.bass_utils import run_bass_kernel_spmd  # noqa: E402

F32 = mybir.dt.float32
F32R = mybir.dt.float32r
BF16 = mybir.dt.bfloat16
I32 = mybir.dt.int32

ROPE_THETA = 10000.0
D = 128  # head dim

_NC_CACHE = {}


def build_nc(S=2048, HID=4096, NQ=8, NKV=2, rope_f32r=True):
    REP = NQ // NKV           # q heads per kv head (4)
    QC = NQ * D               # q cols per core (1024)
    KC = NKV * D              # k (or v) cols per core (256)
    NQK = NQ + NKV            # q+k col-tiles (10)
    NHT = HID // 128          # hidden-dim tiles (32)
    CHUNK = 512
    NCH = S // CHUNK          # 4
    SLAB = 512
    NSLAB = S // SLAB         # 4
    SKT = SLAB // 128         # 4
    NTT = S // 128            # 16 token tiles
    SCALE = float(D) ** -0.5
    RG = [[0, 1, 2, 3], [4, 5, 6, 7]]
    RDT = F32R if rope_f32r else F32

    MAGIC = 12582912.0        # 1.5 * 2**23: float32 round-to-nearest-int trick
    TWOPI = float(2.0 * np.pi)
    INV2PI = float(1.0 / TWOPI)
    HALFPI = float(np.pi / 2.0)

    nc = bacc.Bacc(None, target_bir_lowering=False)
    hidden = nc.declare_dram_parameter("hidden_states", [S, HID], F32, isOutput=False)
    positions = nc.declare_dram_parameter("positions", [1, S], I32, isOutput=False)
    w_qkv = nc.declare_dram_parameter("w_qkv", [HID, QC + 2 * KC], F32, isOutput=False)
    w_o = nc.declare_dram_parameter("w_o", [QC, HID], F32, isOutput=False)
    invf = nc.declare_dram_parameter("invfreq128", [128, 1], F32, isOutput=False)
    masks = nc.declare_dram_parameter("masks", [128, SKT, SLAB], BF16, isOutput=False)
    swapm = nc.declare_dram_parameter("swapmat", [128, 128], RDT, isOutput=False)
    # out chunk idx = s*4 + tt ; rows = this core's 32-token share of that chunk
    out = nc.declare_dram_parameter("out", [NSLAB * SKT, SLAB // 16, HID], F32,
                                    isOutput=True)

    Exp = mybir.ActivationFunctionType.Exp
    Sin = mybir.ActivationFunctionType.Sin
    mul_op = mybir.AluOpType.mult
    add_op = mybir.AluOpType.add

    with tile.TileContext(nc) as tc:
      with tc.tile_pool(name="dram", bufs=1, space="DRAM") as dram:
        hid_bf = dram.tile([S, HID], BF16, name="hid_bf", tag="hid_bf")
        qkT = dram.tile([NQK, 128, S], BF16, name="qkT", tag="qkT")
        v_dr = dram.tile([NTT, 128, KC], BF16, name="v_dr", tag="v_dr")
        parts = [dram.tile([128, HID], BF16, name=f"part{i}", tag=f"part{i}")
                 for i in range(NSLAB * SKT)]
        rsouts = [dram.tile([SLAB // 16, HID], BF16, name=f"rsout{i}",
                            tag=f"rsout{i}")
                  for i in range(NSLAB * SKT)]

        with tc.tile_pool(name="const", bufs=1) as cpool:
            invf_sb = cpool.tile([128, 1], F32, name="invf", tag="invf")
            nc.sync.dma_start(invf_sb[:], invf[:])
            ones_col = cpool.tile([128, 1], BF16, name="ones_col", tag="ones_col")
            nc.vector.memset(ones_col[:], 1.0)
            ones_row = cpool.tile([1, 128], F32, name="ones_row", tag="ones_row")
            nc.vector.memset(ones_row[:], 1.0)
            swap_sb = cpool.tile([128, 128], RDT, name="swapm", tag="swapm")
            nc.sync.dma_start(swap_sb[:], swapm[:])
            cosc = cpool.tile([128, S], BF16, name="cosc", tag="cosc")
            sinc = cpool.tile([128, S], BF16, name="sinc", tag="sinc")

            # ---- trig precompute for the whole sequence (once) ----
            with (
                tc.tile_pool(name="trig", bufs=1) as tgp,
                tc.tile_pool(name="psTR", bufs=1, space="PSUM") as ptr,
            ):
                pos_i = tgp.tile([1, S], I32, name="posi", tag="posi")
                nc.sync.dma_start(pos_i[:], positions[0:1, :])
                pos_c = tgp.tile([1, S], F32, name="posc", tag="posc")
                nc.vector.tensor_copy(pos_c[:], pos_i[:])
                ppos = ptr.tile([128, S], F32, name="ppos", tag="ppos")
                for j in range(S // 512):
                    nc.tensor.matmul(ppos[:, 512 * j:512 * (j + 1)], ones_row[:],
                                     pos_c[:, 512 * j:512 * (j + 1)],
                                     start=True, stop=True)
                # signed angle: invf rows 0..63 are negative -> sin gets the
                # sign for free, cos is even so unaffected.
                ang = tgp.tile([128, S], F32, name="ang", tag="ang")
                nc.vector.tensor_scalar_mul(ang[:], ppos[:], invf_sb[:])
                tmp = tgp.tile([128, S], F32, name="ttmp", tag="ttmp")
                red = tgp.tile([128, S], F32, name="tred", tag="tred")
                for dst, phase in ((cosc, HALFPI), (sinc, 0.0)):
                    nc.vector.tensor_scalar(
                        tmp[:], ang[:], INV2PI, phase * INV2PI,
                        op0=mul_op, op1=add_op)
                    nc.vector.tensor_scalar_add(tmp[:], tmp[:], MAGIC)
                    nc.vector.tensor_scalar_sub(tmp[:], tmp[:], MAGIC)
                    nc.vector.scalar_tensor_tensor(
                        red[:], tmp[:], -TWOPI, ang[:],
                        op0=mul_op, op1=add_op)
                    if phase != 0.0:
                        nc.vector.tensor_scalar_add(red[:], red[:], phase)
                    nc.vector.tensor_scalar_min(red[:], red[:], 3.141592)
                    nc.vector.tensor_scalar_max(red[:], red[:], -3.141592)
                    nc.scalar.activation(dst[:], red[:], Sin)

            # ================= A0: QKV + RoPE =================
            with (
                tc.tile_pool(name="wq", bufs=1) as wqp,
                tc.tile_pool(name="hidT", bufs=2) as hTp,
                tc.tile_pool(name="rope", bufs=3) as rp,
                tc.tile_pool(name="vw", bufs=2) as vwp,
                tc.tile_pool(name="psA", bufs=3, space="PSUM") as psA,
                tc.tile_pool(name="psW", bufs=2, space="PSUM") as psW,
                tc.tile_pool(name="psV", bufs=2, space="PSUM") as psV,
            ):
                wq_sb = [wqp.tile([128, QC + 2 * KC], BF16,
                                  name=f"wq{h}", tag=f"wq{h}")
                         for h in range(NHT)]
                # SWDGE order: a few weight tiles first (first MMs need low h),
                # then the chunk casts interleaved with the rest.
                for h in range(4):
                    nc.gpsimd.dma_start(
                        wq_sb[h][:], w_qkv[128 * h:128 * (h + 1), :])
                nc.gpsimd.dma_start(hid_bf[0:CHUNK, :], hidden[0:CHUNK, :])
                for h in range(4, 16):
                    nc.gpsimd.dma_start(
                        wq_sb[h][:], w_qkv[128 * h:128 * (h + 1), :])
                nc.gpsimd.dma_start(hid_bf[CHUNK:2 * CHUNK, :],
                                    hidden[CHUNK:2 * CHUNK, :])
                for h in range(16, NHT):
                    nc.gpsimd.dma_start(
                        wq_sb[h][:], w_qkv[128 * h:128 * (h + 1), :])
                for c in range(2, NCH):
                    nc.gpsimd.dma_start(hid_bf[CHUNK * c:CHUNK * (c + 1), :],
                                        hidden[CHUNK * c:CHUNK * (c + 1), :])

                for c in range(NCH):
                    c0 = CHUNK * c
                    hidT = [hTp.tile([128, CHUNK], BF16,
                                     name=f"hidT{h}", tag=f"hidT{h}")
                            for h in range(NHT)]
                    for h in range(NHT):
                        nc.sync.dma_start_transpose(
                            hidT[h][:], hid_bf[c0:c0 + CHUNK,
                                               128 * h:128 * (h + 1)])
                    for ct in range(NQK):
                        pq = psA.tile([128, CHUNK], F32, name="pq", tag="pq")
                        for h in range(NHT):
                            nc.tensor.matmul(
                                pq[:],
                                wq_sb[h][:, 128 * ct:128 * (ct + 1)],
                                hidT[h][:],
                                start=(h == 0), stop=(h == NHT - 1),
                            )
                        qw = rp.tile([128, CHUNK], RDT, name="qw", tag="qw")
                        nc.scalar.copy(qw[:], pq[:])
                        pswap = psW.tile([128, CHUNK], F32, name="pswap",
                                         tag="pswap")
                        nc.tensor.matmul(pswap[:], swap_sb[:], qw[:],
                                         start=True, stop=True)
                        qcos = rp.tile([128, CHUNK], BF16, name="qcos",
                                       tag="qcos")
                        nc.vector.tensor_mul(qcos[:], qw[:],
                                             cosc[:, c0:c0 + CHUNK])
                        rot = rp.tile([128, CHUNK], BF16, name="rot", tag="rot")
                        nc.vector.tensor_mul(rot[:], pswap[:],
                                             sinc[:, c0:c0 + CHUNK])
                        qout = rp.tile([128, CHUNK], BF16, name="qout",
                                       tag="qout")
                        nc.vector.tensor_add(qout[:], qcos[:], rot[:])
                        nc.sync.dma_start(qkT[ct][:, c0:c0 + CHUNK], qout[:])
                    for tt in range(CHUNK // 128):
                        pv = psV.tile([128, KC], F32, name="pv", tag="pv")
                        for h in range(NHT):
                            nc.tensor.matmul(
                                pv[:],
                                hidT[h][:, 128 * tt:128 * (tt + 1)],
                                wq_sb[h][:, QC + KC:QC + 2 * KC],
                                start=(h == 0), stop=(h == NHT - 1),
                            )
                        vw = vwp.tile([128, KC], BF16, name="vw", tag="vw")
                        nc.scalar.copy(vw[:], pv[:])
                        nc.sync.dma_start(v_dr[c * (CHUNK // 128) + tt], vw[:])

            # ============ A1: attention + o-proj + RS ============
            with (
                tc.tile_pool(name="qT", bufs=1) as qTp,
                tc.tile_pool(name="kT", bufs=1) as kTp,
                tc.tile_pool(name="vsb", bufs=1) as vp,
                tc.tile_pool(name="wo", bufs=1) as wop,
                tc.tile_pool(name="maskp", bufs=1) as mkp,
                tc.tile_pool(name="at", bufs=2) as atp,
                tc.tile_pool(name="den", bufs=2) as dnp,
                tc.tile_pool(name="pt", bufs=3) as ptp,
                tc.tile_pool(name="bcp", bufs=2) as bcp,
                tc.tile_pool(name="ot", bufs=3) as otp,
                tc.tile_pool(name="psS", bufs=2, space="PSUM") as psS,
                tc.tile_pool(name="psPV", bufs=2, space="PSUM") as psPV,
                tc.tile_pool(name="psX", bufs=1, space="PSUM") as psX,
                tc.tile_pool(name="psO", bufs=1, space="PSUM") as psO,
            ):
                kT_sb = [kTp.tile([128, S], BF16, name=f"kT{i}", tag=f"kT{i}")
                         for i in range(NKV)]
                for i in range(NKV):
                    nc.sync.dma_start(kT_sb[i][:], qkT[NQ + i])
                v_sb = [vp.tile([128, KC], BF16, name=f"v{t}", tag=f"v{t}")
                        for t in range(NTT)]
                for t in range(NTT):
                    nc.sync.dma_start(v_sb[t][:], v_dr[t])
                qT_sb = [qTp.tile([128, S], BF16, name=f"qT{i}", tag=f"qT{i}")
                         for i in range(NQ)]
                for i in range(NQ):
                    nc.sync.dma_start(qT_sb[i][:], qkT[i])
                mask_sb = mkp.tile([128, SKT, SLAB], BF16, name="masks",
                                   tag="masks")
                nc.sync.dma_start(mask_sb[:], masks[:])
                wo_sb = [wop.tile([128, HID], BF16, name=f"wo{q}", tag=f"wo{q}")
                         for q in range(NQ)]
                for q in range(NQ):
                    nc.gpsimd.dma_start(wo_sb[q][:],
                                        w_o[128 * q:128 * (q + 1), :])

                def oproj_piece(s, pi, ats):
                    tt, hc = divmod(pi, HID // 512)
                    po = psO.tile([128, 512], F32, name="po", tag="po")
                    for q in range(NQ):
                        nc.tensor.matmul(
                            po[:],
                            ats[q][:, 128 * tt:128 * (tt + 1)],
                            wo_sb[q][:, 512 * hc:512 * (hc + 1)],
                            start=(q == 0), stop=(q == NQ - 1),
                        )
                    ot = otp.tile([128, 512], BF16, name="ot", tag="ot")
                    nc.scalar.copy(ot[:], po[:])
                    idx = SKT * s + tt
                    nc.sync.dma_start(parts[idx][:, 512 * hc:512 * (hc + 1)],
                                      ot[:])
                    if hc == HID // 512 - 1:
                        nc.gpsimd.collective_compute(
                            "ReduceScatter",
                            mybir.AluOpType.add,
                            replica_groups=RG,
                            ins=[parts[idx].opt()],
                            outs=[rsouts[idx].opt()],
                        )
                        nc.gpsimd.dma_start(out[idx], rsouts[idx][:])

                at_prev = None
                for s in range(NSLAB):
                    s0 = SLAB * s
                    NSUP = 2 * (s + 1)
                    at_cur = []
                    for hq in range(NQ):
                        kvh = hq // REP
                        ppv = psPV.tile([128, SLAB], F32, name="ppv", tag="ppv")
                        den = dnp.tile([128, 2, 512], BF16, name="den",
                                       tag="den")
                        den_src = None
                        for j in range(NSUP):
                            ps = psS.tile([128, 2, 512], F32, name="ps",
                                          tag="ps")
                            for u in range(2):
                                kt = 2 * j + u
                                nc.tensor.matmul(
                                    ps[:, u, :],
                                    kT_sb[kvh][:, 128 * kt:128 * (kt + 1)],
                                    qT_sb[hq][:, s0:s0 + SLAB],
                                    start=True, stop=True,
                                )
                            pt = ptp.tile([128, 2, 512], BF16, name="pt",
                                          tag="pt")
                            nc.scalar.activation(pt[:], ps[:], Exp, scale=SCALE)
                            dj = j - 2 * s
                            if 0 <= dj <= 1:
                                nc.vector.tensor_mul(
                                    pt[:], pt[:],
                                    mask_sb[:, 2 * dj:2 * dj + 2, :])
                            if NSUP > 1:
                                if j == 0:
                                    nc.vector.tensor_copy(den[:], pt[:])
                                else:
                                    nc.vector.tensor_add(den[:], den[:], pt[:])
                                den_src = den
                            else:
                                den_src = pt
                            for u in range(2):
                                kt = 2 * j + u
                                nc.tensor.matmul(
                                    ppv[:],
                                    v_sb[kt][:, D * kvh:D * (kvh + 1)],
                                    pt[:, u, :],
                                    start=(j == 0 and u == 0),
                                    stop=(j == NSUP - 1 and u == 1),
                                )
                        pden = psX.tile([128, 512], F32, name="pden",
                                        tag="pden")
                        for u in range(2):
                            nc.tensor.matmul(pden[0:1, :], ones_col[:],
                                             den_src[:, u, :],
                                             start=(u == 0), stop=(u == 1))
                        rec = bcp.tile([1, 512], F32, name="rec", tag="rec")
                        nc.vector.reciprocal_approx_fast(rec[:], pden[0:1, :])
                        pbc = psX.tile([128, 512], F32, name="pbc", tag="pden")
                        nc.tensor.matmul(pbc[:], ones_row[:], rec[:],
                                         start=True, stop=True)
                        bc = bcp.tile([128, 512], F32, name="bc", tag="bc")
                        nc.vector.tensor_copy(bc[:], pbc[:])
                        at = atp.tile([128, SLAB], BF16,
                                      name=f"at{hq}", tag=f"at{hq}")
                        nc.vector.tensor_mul(at[:], ppv[:], bc[:])
                        at_cur.append(at)
                        if at_prev is not None:
                            for pi in range(4 * hq, 4 * hq + 4):
                                oproj_piece(s - 1, pi, at_prev)
                    at_prev = at_cur
                for pi in range(4 * NQ):
                    oproj_piece(NSLAB - 1, pi, at_prev)

    nc.compile()
    return nc


def make_consts(S=2048):
    SLAB = min(512, S)
    SKT = SLAB // 128
    d_half = np.arange(0, D, 2, dtype=np.float32) / D
    invfreq = (1.0 / (ROPE_THETA ** d_half)).astype(np.float32)  # [64]
    # signed: rows 0..63 negative (sin sign trick), cos unaffected (even fn)
    invf128 = np.concatenate([-invfreq, invfreq]).reshape(128, 1).astype(np.float32)
    p = np.arange(128).reshape(128, 1, 1)
    j = np.arange(SKT).reshape(1, SKT, 1)
    q = np.arange(SLAB).reshape(1, 1, SLAB)
    masks = ((j * 128 + p) <= q).astype(ml_dtypes.bfloat16)  # [128, SKT, SLAB]
    swapmat = np.zeros((128, 128), np.float32)
    for pp in range(128):
        swapmat[pp, (pp + 64) % 128] = 1.0
    return invf128, masks, swapmat


def shard_inputs(hidden_states, positions, w_qkv, w_o, n_q_total=32, n_kv_total=8,
                 tp=4):
    """Returns in_maps for 8 cores: DP over batch x TP over heads."""
    B, S, HIDDEN = hidden_states.shape
    q_size = n_q_total * D
    kv_size = n_kv_total * D
    nq = n_q_total // tp           # q heads per core
    nkv = n_kv_total // tp         # kv heads per core
    invf128, masks, swapmat = make_consts(S=S)
    in_maps = []
    for c in range(8):
        g, r = divmod(c, tp)
        wq = w_qkv[:, nq * D * r: nq * D * (r + 1)]
        wk = w_qkv[:, q_size + nkv * D * r: q_size + nkv * D * (r + 1)]
        wv = w_qkv[:, q_size + kv_size + nkv * D * r: q_size + kv_size + nkv * D * (r + 1)]
        in_maps.append({
            "hidden_states": np.ascontiguousarray(hidden_states[g]),
            "positions": np.ascontiguousarray(positions[g:g + 1]).astype(np.int32),
            "w_qkv": np.ascontiguousarray(np.concatenate([wq, wk, wv], axis=1)),
            "w_o": np.ascontiguousarray(w_o[nq * D * r: nq * D * (r + 1), :]),
            "invfreq128": invf128,
            "masks": masks,
            "swapmat": swapmat,
        })
    return in_maps


def assemble_output(results, B=2, S=2048, HIDDEN=4096, tp=4):
    SLAB = min(512, S)
    NSLAB = S // SLAB
    SKT = SLAB // 128
    RSROWS = SLAB // 16    # 32 rows per (chunk, core)
    out = np.empty((B, S, HIDDEN), dtype=np.float32)
    for c in range(8):
        g, r = divmod(c, tp)
        o = np.asarray(results[c]["out"]).reshape(NSLAB * SKT, RSROWS, HIDDEN)
        for s in range(NSLAB):
            for tt in range(SKT):
                t0 = SLAB * s + 128 * tt + RSROWS * r
                out[g, t0:t0 + RSROWS, :] = o[SKT * s + tt]
    return out


def kernel(hidden_states, positions, w_qkv, w_o):
    hidden_states = np.asarray(hidden_states, dtype=np.float32)
    positions = np.asarray(positions, dtype=np.int32)
    w_qkv = np.asarray(w_qkv, dtype=np.float32)
    w_o = np.asarray(w_o, dtype=np.float32)
    B, S, HIDDEN = hidden_states.shape

    key = (S, HIDDEN)
    if key not in _NC_CACHE:
        try:
            _NC_CACHE[key] = build_nc(S=S, HID=HIDDEN, rope_f32r=True)
        except Exception:
            _NC_CACHE[key] = build_nc(S=S, HID=HIDDEN, rope_f32r=False)
    nc = _NC_CACHE[key]

    in_maps = shard_inputs(hidden_states, positions, w_qkv, w_o)
    res = run_bass_kernel_spmd(nc, in_maps, core_ids=list(range(8)))
    return assemble_output(res.results, B=B, S=S, HIDDEN=HIDDEN)


if __name__ == "__main__":
    rng = np.random.default_rng(0)
    B, S, HIDDEN = 2, 2048, 4096
    hs = rng.standard_normal((B, S, HIDDEN), dtype=np.float32)
    pos = np.arange(B * S, dtype=np.int32).reshape(B, S)
    wq = rng.standard_normal((HIDDEN, 6144), dtype=np.float32) * HIDDEN ** -0.5
    wo = rng.standard_normal((4096, HIDDEN), dtype=np.float32) * 4096 ** -0.5
    o = kernel(hs, pos, wq, wo)
    print(o.shape, o.dtype)


# revision 9
# speedup vs baseline: 1.1964x; 1.1755x over previous
"""Distributed Trainium2 kernel for ArceeAttention (GQA + RoPE + causal attention).

Sharding: DP over batch (2 groups of 4 cores) x TP-4 over heads within each group.
Each core: 8 q heads + 2 kv heads, full sequence of its batch.

Structure (v2 — engine-dense rework of the baseline):
  A0: hidden -> bf16 DRAM staging -> xbar transposes [512,128] on BOTH HWDGE
      queues (sync+scalar); chunk-major QKV with N=512 moving operand and the
      full w_qkv resident bf16; RoPE trig precomputed once for the whole
      sequence (sign folded into invfreq); RoPE swap matmul in float32r
      (1 cyc/row); qT/kT/v staged to DRAM (keeps SBUF under budget).
  A1: per 512-token slab: scores in [128, 2x512] PSUM supertiles, ONE exp per
      supertile, denominator accumulated on DVE as bf16 supertiles and folded
      by ones-matmuls; o-proj of slab s-1 interleaved with attention of slab s
      so the PE never drains (HAM stays warm) while ACT chews exps;
      ReduceScatter chunked per 128 tokens (16 ops) so the tail is short.
"""
import sys
import numpy as np

for _p in ("/opt/trn_rl_repo",):
    if _p not in sys.path:
        sys.path.append(_p)

import ml_dtypes  # noqa: E402
from concourse import bass, bacc, tile, mybir  # noqa: E402
from concourse.bass_utils import run_bass_kernel_spmd  # noqa: E402

F32 = mybir.dt.float32
F32R = mybir.dt.float32r
BF16 = mybir.dt.bfloat16
I32 = mybir.dt.int32

ROPE_THETA = 10000.0
D = 128  # head dim

_NC_CACHE = {}


def build_nc(S=2048, HID=4096, NQ=8, NKV=2, rope_f32r=True):
    REP = NQ // NKV           # q heads per kv head (4)
    QC = NQ * D               # q cols per core (1024)
    KC = NKV * D              # k (or v) cols per core (256)
    NQK = NQ + NKV            # q+k col-tiles (10)
    NHT = HID // 128          # hidden-dim tiles (32)
    CHUNK = 512
    NCH = S // CHUNK          # 4
    SLAB = 512
    NSLAB = S // SLAB         # 4
    SKT = SLAB // 128         # 4
    NTT = S // 128            # 16 token tiles
    SCALE = float(D) ** -0.5
    RG = [[0, 1, 2, 3], [4, 5, 6, 7]]
    RDT = F32R if rope_f32r else F32

    MAGIC = 12582912.0        # 1.5 * 2**23: float32 round-to-nearest-int trick
    TWOPI = float(2.0 * np.pi)
    INV2PI = float(1.0 / TWOPI)
    HALFPI = float(np.pi / 2.0)

    nc = bacc.Bacc(None, target_bir_lowering=False)
    hidden = nc.declare_dram_parameter("hidden_states", [S, HID], BF16, isOutput=False)
    positions = nc.declare_dram_parameter("positions", [1, S], I32, isOutput=False)
    w_qkv = nc.declare_dram_parameter("w_qkv", [HID, QC + 2 * KC], BF16, isOutput=False)
    w_o = nc.declare_dram_parameter("w_o", [QC, HID], BF16, isOutput=False)
    invf = nc.declare_dram_parameter("invfreq128", [128, 1], F32, isOutput=False)
    masks = nc.declare_dram_parameter("masks", [128, SKT, SLAB], BF16, isOutput=False)
    swapm = nc.declare_dram_parameter("swapmat", [128, 128], RDT, isOutput=False)
    # out chunk idx = s*4 + tt ; rows = this core's 32-token share of that chunk
    out = nc.declare_dram_parameter("out", [NSLAB * SKT, SLAB // 16, HID], F32,
                                    isOutput=True)

    Exp = mybir.ActivationFunctionType.Exp
    Sin = mybir.ActivationFunctionType.Sin
    mul_op = mybir.AluOpType.mult
    add_op = mybir.AluOpType.add

    with tile.TileContext(nc) as tc:
      with tc.tile_pool(name="dram", bufs=1, space="DRAM") as dram:
        qkT = dram.tile([NQK, 128, S], BF16, name="qkT", tag="qkT")
        v_dr = dram.tile([NTT, 128, KC], BF16, name="v_dr", tag="v_dr")
        parts = [dram.tile([128, HID], BF16, name=f"part{i}", tag=f"part{i}")
                 for i in range(NSLAB * SKT)]
        rsouts = [dram.tile([SLAB // 16, HID], BF16, name=f"rsout{i}",
                            tag=f"rsout{i}")
                  for i in range(NSLAB * SKT)]

        with tc.tile_pool(name="const", bufs=1) as cpool:
            invf_sb = cpool.tile([128, 1], F32, name="invf", tag="invf")
            nc.sync.dma_start(invf_sb[:], invf[:])
            ones_col = cpool.tile([128, 1], BF16, name="ones_col", tag="ones_col")
            nc.vector.memset(ones_col[:], 1.0)
            ones_row = cpool.tile([1, 128], F32, name="ones_row", tag="ones_row")
            nc.vector.memset(ones_row[:], 1.0)
            swap_sb = cpool.tile([128, 128], RDT, name="swapm", tag="swapm")
            nc.sync.dma_start(swap_sb[:], swapm[:])
            cosc = cpool.tile([128, S], BF16, name="cosc", tag="cosc")
            sinc = cpool.tile([128, S], BF16, name="sinc", tag="sinc")

            # ---- trig precompute for the whole sequence (once) ----
            with (
                tc.tile_pool(name="trig", bufs=1) as tgp,
                tc.tile_pool(name="psTR", bufs=1, space="PSUM") as ptr,
            ):
                pos_i = tgp.tile([1, S], I32, name="posi", tag="posi")
                nc.sync.dma_start(pos_i[:], positions[0:1, :])
                pos_c = tgp.tile([1, S], F32, name="posc", tag="posc")
                nc.vector.tensor_copy(pos_c[:], pos_i[:])
                ppos = ptr.tile([128, S], F32, name="ppos", tag="ppos")
                for j in range(S // 512):
                    nc.tensor.matmul(ppos[:, 512 * j:512 * (j + 1)], ones_row[:],
                                     pos_c[:, 512 * j:512 * (j + 1)],
                                     start=True, stop=True)
                # signed angle: invf rows 0..63 are negative -> sin gets the
                # sign for free, cos is even so unaffected.
                ang = tgp.tile([128, S], F32, name="ang", tag="ang")
                nc.vector.tensor_scalar_mul(ang[:], ppos[:], invf_sb[:])
                tmp = tgp.tile([128, S], F32, name="ttmp", tag="ttmp")
                red = tgp.tile([128, S], F32, name="tred", tag="tred")
                for dst, phase in ((cosc, HALFPI), (sinc, 0.0)):
                    nc.vector.tensor_scalar(
                        tmp[:], ang[:], INV2PI, phase * INV2PI,
                        op0=mul_op, op1=add_op)
                    nc.vector.tensor_scalar_add(tmp[:], tmp[:], MAGIC)
                    nc.vector.tensor_scalar_sub(tmp[:], tmp[:], MAGIC)
                    nc.vector.scalar_tensor_tensor(
                        red[:], tmp[:], -TWOPI, ang[:],
                        op0=mul_op, op1=add_op)
                    if phase != 0.0:
                        nc.vector.tensor_scalar_add(red[:], red[:], phase)
                    nc.vector.tensor_scalar_min(red[:], red[:], 3.141592)
                    nc.vector.tensor_scalar_max(red[:], red[:], -3.141592)
                    nc.scalar.activation(dst[:], red[:], Sin)

            # ================= A0: QKV + RoPE =================
            with (
                tc.tile_pool(name="wq", bufs=1) as wqp,
                tc.tile_pool(name="hidT", bufs=2) as hTp,
                tc.tile_pool(name="rope", bufs=3) as rp,
                tc.tile_pool(name="vw", bufs=2) as vwp,
                tc.tile_pool(name="psA", bufs=3, space="PSUM") as psA,
                tc.tile_pool(name="psW", bufs=2, space="PSUM") as psW,
                tc.tile_pool(name="psV", bufs=2, space="PSUM") as psV,
            ):
                wq_sb = [wqp.tile([128, QC + 2 * KC], BF16,
                                  name=f"wq{h}", tag=f"wq{h}")
                         for h in range(NHT)]
                # weights on the SWDGE ring (plain bf16 copies) so the sync
                # HWDGE ring is free for the xbar transposes from t=0.
                for h in range(NHT):
                    nc.gpsimd.dma_start(
                        wq_sb[h][:], w_qkv[128 * h:128 * (h + 1), :])

                for c in range(NCH):
                    c0 = CHUNK * c
                    hidT = [hTp.tile([128, CHUNK], BF16,
                                     name=f"hidT{h}", tag=f"hidT{h}")
                            for h in range(NHT)]
                    for h in range(NHT):
                        nc.sync.dma_start_transpose(
                            hidT[h][:], hidden[c0:c0 + CHUNK,
                                               128 * h:128 * (h + 1)])
                    for ct in range(NQK):
                        pq = psA.tile([128, CHUNK], F32, name="pq", tag="pq")
                        for h in range(NHT):
                            nc.tensor.matmul(
                                pq[:],
                                wq_sb[h][:, 128 * ct:128 * (ct + 1)],
                                hidT[h][:],
                                start=(h == 0), stop=(h == NHT - 1),
                            )
                        qw = rp.tile([128, CHUNK], RDT, name="qw", tag="qw")
                        nc.scalar.copy(qw[:], pq[:])
                        pswap = psW.tile([128, CHUNK], F32, name="pswap",
                                         tag="pswap")
                        nc.tensor.matmul(pswap[:], swap_sb[:], qw[:],
                                         start=True, stop=True)
                        qcos = rp.tile([128, CHUNK], BF16, name="qcos",
                                       tag="qcos")
                        nc.vector.tensor_mul(qcos[:], qw[:],
                                             cosc[:, c0:c0 + CHUNK])
                        rot = rp.tile([128, CHUNK], BF16, name="rot", tag="rot")
                        nc.vector.tensor_mul(rot[:], pswap[:],
                                             sinc[:, c0:c0 + CHUNK])
                        qout = rp.tile([128, CHUNK], BF16, name="qout",
                                       tag="qout")
                        nc.vector.tensor_add(qout[:], qcos[:], rot[:])
                        nc.sync.dma_start(qkT[ct][:, c0:c0 + CHUNK], qout[:])
                    for tt in range(CHUNK // 128):
                        pv = psV.tile([128, KC], F32, name="pv", tag="pv")
                        for h in range(NHT):
                            nc.tensor.matmul(
                                pv[:],
                                hidT[h][:, 128 * tt:128 * (tt + 1)],
                                wq_sb[h][:, QC + KC:QC + 2 * KC],
                                start=(h == 0), stop=(h == NHT - 1),
                            )
                        vw = vwp.tile([128, KC], BF16, name="vw", tag="vw")
                        nc.scalar.copy(vw[:], pv[:])
                        nc.sync.dma_start(v_dr[c * (CHUNK // 128) + tt], vw[:])

            # ============ A1: attention + o-proj + RS ============
            with (
                tc.tile_pool(name="qT", bufs=1) as qTp,
                tc.tile_pool(name="kT", bufs=1) as kTp,
                tc.tile_pool(name="vsb", bufs=1) as vp,
                tc.tile_pool(name="wo", bufs=1) as wop,
                tc.tile_pool(name="maskp", bufs=1) as mkp,
                tc.tile_pool(name="at", bufs=2) as atp,
                tc.tile_pool(name="den", bufs=2) as dnp,
                tc.tile_pool(name="pt", bufs=3) as ptp,
                tc.tile_pool(name="bcp", bufs=2) as bcp,
                tc.tile_pool(name="ot", bufs=3) as otp,
                tc.tile_pool(name="psS", bufs=2, space="PSUM") as psS,
                tc.tile_pool(name="psPV", bufs=2, space="PSUM") as psPV,
                tc.tile_pool(name="psX", bufs=1, space="PSUM") as psX,
                tc.tile_pool(name="psO", bufs=1, space="PSUM") as psO,
            ):
                kT_sb = [kTp.tile([128, S], BF16, name=f"kT{i}", tag=f"kT{i}")
                         for i in range(NKV)]
                v_sb = [vp.tile([128, KC], BF16, name=f"v{t}", tag=f"v{t}")
                        for t in range(NTT)]
                qT_sb = [qTp.tile([128, S], BF16, name=f"qT{i}", tag=f"qT{i}")
                         for i in range(NQ)]
                mask_sb = mkp.tile([128, SKT, SLAB], BF16, name="masks",
                                   tag="masks")
                wo_sb = [wop.tile([128, HID], BF16, name=f"wo{q}", tag=f"wo{q}")
                         for q in range(NQ)]
                # load order tuned so slab-0/head-0 can start ASAP
                for i in range(NKV):
                    nc.sync.dma_start(kT_sb[i][:], qkT[NQ + i])
                nc.sync.dma_start(qT_sb[0][:], qkT[0])
                nc.sync.dma_start(mask_sb[:], masks[:])
                for t in range(SKT):
                    nc.sync.dma_start(v_sb[t][:], v_dr[t])
                for i in range(1, NQ):
                    nc.sync.dma_start(qT_sb[i][:], qkT[i])
                for t in range(SKT, NTT):
                    nc.sync.dma_start(v_sb[t][:], v_dr[t])
                for q in range(NQ):
                    nc.gpsimd.dma_start(wo_sb[q][:],
                                        w_o[128 * q:128 * (q + 1), :])

                def oproj_piece(s, pi, ats):
                    tt, hc = divmod(pi, HID // 512)
                    po = psO.tile([128, 512], F32, name="po", tag="po")
                    for q in range(NQ):
                        nc.tensor.matmul(
                            po[:],
                            ats[q][:, 128 * tt:128 * (tt + 1)],
                            wo_sb[q][:, 512 * hc:512 * (hc + 1)],
                            start=(q == 0), stop=(q == NQ - 1),
                        )
                    ot = otp.tile([128, 512], BF16, name="ot", tag="ot")
                    nc.scalar.copy(ot[:], po[:])
                    idx = SKT * s + tt
                    nc.sync.dma_start(parts[idx][:, 512 * hc:512 * (hc + 1)],
                                      ot[:])
                    if hc == HID // 512 - 1:
                        nc.gpsimd.collective_compute(
                            "ReduceScatter",
                            mybir.AluOpType.add,
                            replica_groups=RG,
                            ins=[parts[idx].opt()],
                            outs=[rsouts[idx].opt()],
                        )
                        nc.gpsimd.dma_start(out[idx], rsouts[idx][:])

                at_prev = None
                for s in range(NSLAB):
                    s0 = SLAB * s
                    NSUP = 2 * (s + 1)
                    sup_total = NQ * NSUP
                    sup_done = 0
                    pieces_emitted = 0
                    at_cur = []
                    for hq in range(NQ):
                        kvh = hq // REP
                        ppv = psPV.tile([128, SLAB], F32, name="ppv", tag="ppv")
                        den = dnp.tile([128, 2, 512], BF16, name="den",
                                       tag="den")
                        den_src = None
                        for j in range(NSUP):
                            ps = psS.tile([128, 2, 512], F32, name="ps",
                                          tag="ps")
                            for u in range(2):
                                kt = 2 * j + u
                                nc.tensor.matmul(
                                    ps[:, u, :],
                                    kT_sb[kvh][:, 128 * kt:128 * (kt + 1)],
                                    qT_sb[hq][:, s0:s0 + SLAB],
                                    start=True, stop=True,
                                )
                            pt = ptp.tile([128, 2, 512], BF16, name="pt",
                                          tag="pt")
                            nc.scalar.activation(pt[:], ps[:], Exp, scale=SCALE)
                            dj = j - 2 * s
                            if 0 <= dj <= 1:
                                nc.vector.tensor_mul(
                                    pt[:], pt[:],
                                    mask_sb[:, 2 * dj:2 * dj + 2, :])
                            if NSUP > 1:
                                if j == 0:
                                    nc.vector.tensor_copy(den[:], pt[:])
                                else:
                                    nc.vector.tensor_add(den[:], den[:], pt[:])
                                den_src = den
                            else:
                                den_src = pt
                            # fill the QK->exp->PV latency with o-proj MMs of
                            # the previous slab (keeps the PE dense and warm)
                            sup_done += 1
                            if at_prev is not None:
                                due = (4 * NQ * sup_done) // sup_total
                                while pieces_emitted < due:
                                    oproj_piece(s - 1, pieces_emitted, at_prev)
                                    pieces_emitted += 1
                            for u in range(2):
                                kt = 2 * j + u
                                nc.tensor.matmul(
                                    ppv[:],
                                    v_sb[kt][:, D * kvh:D * (kvh + 1)],
                                    pt[:, u, :],
                                    start=(j == 0 and u == 0),
                                    stop=(j == NSUP - 1 and u == 1),
                                )
                        pden = psX.tile([128, 512], F32, name="pden",
                                        tag="pden")
                        for u in range(2):
                            nc.tensor.matmul(pden[0:1, :], ones_col[:],
                                             den_src[:, u, :],
                                             start=(u == 0), stop=(u == 1))
                        rec = bcp.tile([1, 512], F32, name="rec", tag="rec")
                        nc.vector.reciprocal_approx_fast(rec[:], pden[0:1, :])
                        pbc = psX.tile([128, 512], F32, name="pbc", tag="pden")
                        nc.tensor.matmul(pbc[:], ones_row[:], rec[:],
                                         start=True, stop=True)
                        bc = bcp.tile([128, 512], F32, name="bc", tag="bc")
                        nc.vector.tensor_copy(bc[:], pbc[:])
                        at = atp.tile([128, SLAB], BF16,
                                      name=f"at{hq}", tag=f"at{hq}")
                        nc.vector.tensor_mul(at[:], ppv[:], bc[:])
                        at_cur.append(at)
                    while at_prev is not None and pieces_emitted < 4 * NQ:
                        oproj_piece(s - 1, pieces_emitted, at_prev)
                        pieces_emitted += 1
                    at_prev = at_cur
                for pi in range(4 * NQ):
                    oproj_piece(NSLAB - 1, pi, at_prev)

    nc.compile()
    return nc


def make_consts(S=2048):
    SLAB = min(512, S)
    SKT = SLAB // 128
    d_half = np.arange(0, D, 2, dtype=np.float32) / D
    invfreq = (1.0 / (ROPE_THETA ** d_half)).astype(np.float32)  # [64]
    # signed: rows 0..63 negative (sin sign trick), cos unaffected (even fn)
    invf128 = np.concatenate([-invfreq, invfreq]).reshape(128, 1).astype(np.float32)
    p = np.arange(128).reshape(128, 1, 1)
    j = np.arange(SKT).reshape(1, SKT, 1)
    q = np.arange(SLAB).reshape(1, 1, SLAB)
    masks = ((j * 128 + p) <= q).astype(ml_dtypes.bfloat16)  # [128, SKT, SLAB]
    swapmat = np.zeros((128, 128), np.float32)
    for pp in range(128):
        swapmat[pp, (pp + 64) % 128] = 1.0
    return invf128, masks, swapmat


def shard_inputs(hidden_states, positions, w_qkv, w_o, n_q_total=32, n_kv_total=8,
                 tp=4):
    """Returns in_maps for 8 cores: DP over batch x TP over heads."""
    B, S, HIDDEN = hidden_states.shape
    q_size = n_q_total * D
    kv_size = n_kv_total * D
    nq = n_q_total // tp           # q heads per core
    nkv = n_kv_total // tp         # kv heads per core
    invf128, masks, swapmat = make_consts(S=S)
    in_maps = []
    for c in range(8):
        g, r = divmod(c, tp)
        wq = w_qkv[:, nq * D * r: nq * D * (r + 1)]
        wk = w_qkv[:, q_size + nkv * D * r: q_size + nkv * D * (r + 1)]
        wv = w_qkv[:, q_size + kv_size + nkv * D * r: q_size + kv_size + nkv * D * (r + 1)]
        in_maps.append({
            "hidden_states": np.ascontiguousarray(
                hidden_states[g]).astype(ml_dtypes.bfloat16),
            "positions": np.ascontiguousarray(positions[g:g + 1]).astype(np.int32),
            "w_qkv": np.ascontiguousarray(
                np.concatenate([wq, wk, wv], axis=1)).astype(ml_dtypes.bfloat16),
            "w_o": np.ascontiguousarray(
                w_o[nq * D * r: nq * D * (r + 1), :]).astype(ml_dtypes.bfloat16),
            "invfreq128": invf128,
            "masks": masks,
            "swapmat": swapmat,
        })
    return in_maps


def assemble_output(results, B=2, S=2048, HIDDEN=4096, tp=4):
    SLAB = min(512, S)
    NSLAB = S // SLAB
    SKT = SLAB // 128
    RSROWS = SLAB // 16    # 32 rows per (chunk, core)
    out = np.empty((B, S, HIDDEN), dtype=np.float32)
    for c in range(8):
        g, r = divmod(c, tp)
        o = np.asarray(results[c]["out"]).reshape(NSLAB * SKT, RSROWS, HIDDEN)
        for s in range(NSLAB):
            for tt in range(SKT):
                t0 = SLAB * s + 128 * tt + RSROWS * r
                out[g, t0:t0 + RSROWS, :] = o[SKT * s + tt]
    return out


def kernel(hidden_states, positions, w_qkv, w_o):
    hidden_states = np.asarray(hidden_states, dtype=np.float32)
    positions = np.asarray(positions, dtype=np.int32)
    w_qkv = np.asarray(w_qkv, dtype=np.float32)
    w_o = np.asarray(w_o, dtype=np.float32)
    B, S, HIDDEN = hidden_states.shape

    key = (S, HIDDEN)
    if key not in _NC_CACHE:
        try:
            _NC_CACHE[key] = build_nc(S=S, HID=HIDDEN, rope_f32r=True)
        except Exception:
            _NC_CACHE[key] = build_nc(S=S, HID=HIDDEN, rope_f32r=False)
    nc = _NC_CACHE[key]

    in_maps = shard_inputs(hidden_states, positions, w_qkv, w_o)
    res = run_bass_kernel_spmd(nc, in_maps, core_ids=list(range(8)))
    return assemble_output(res.results, B=B, S=S, HIDDEN=HIDDEN)


if __name__ == "__main__":
    rng = np.random.default_rng(0)
    B, S, HIDDEN = 2, 2048, 4096
    hs = rng.standard_normal((B, S, HIDDEN), dtype=np.float32)
    pos = np.arange(B * S, dtype=np.int32).reshape(B, S)
    wq = rng.standard_normal((HIDDEN, 6144), dtype=np.float32) * HIDDEN ** -0.5
    wo = rng.standard_normal((4096, HIDDEN), dtype=np.float32) * 4096 ** -0.5
    o = kernel(hs, pos, wq, wo)
    print(o.shape, o.dtype)
